# revision 2
# baseline (speedup 1.0000x reference)
"""ConvLSTM2D cell on 8 Trainium2 NeuronCores — Winograd F(2,3) along H.

Data-parallel over batch: B=16 -> 2 images per core. The 3x3 conv is
computed with 1D Winograd F(2,3) applied along the row (kh) axis: for each
row-pair of outputs, 4 transformed planes t_u (DVE, row-shifted adds) are
contracted against G-transformed weights into 4 PSUM banks m_u (per gate),
and the outputs are recombined y_even = m0+m1+m2, y_odd = m1-m2-m3
(ACT PSUM->SBUF fp16 copies + DVE fp16 adds). This cuts PE row-cycles to
~0.71x of the direct conv (20 vs 28 N=512 matmuls per 16-row gate-chunk:
per u, 3 h-taps at K=128 plus x's kw0+kw1 packed K=128 and kw2 K=64).
ScalarE applies bias+sigmoid/tanh; VectorE does the LSTM elementwise math
in fp16 where possible.
"""

import sys

if "/opt/trn_rl_repo" not in sys.path:
    sys.path.insert(0, "/opt/trn_rl_repo")

import numpy as np

import concourse.bass as bass
import concourse.tile as tile
from concourse import bacc, mybir
from concourse.bass_utils import run_bass_kernel_spmd

N_CORES = 8
B, C_IN, C_HID, H, W = 16, 64, 128, 64, 64
B_LOC = B // N_CORES  # 2 images per core
HP = H + 2  # padded
WP = W + 2
NT = H // 2  # 32 row-tiles per image
TPC = 8  # row-tiles per chunk (16 output rows)
NCHUNK = NT // TPC  # 4 chunks per image

_cache = {}


def _build(dt_mm=mybir.dt.float16, trace=False, unroll=1):
    key = (dt_mm, trace, unroll)
    if key in _cache:
        return _cache[key]
    f32 = mybir.dt.float32
    nc = bacc.Bacc("TRN2", target_bir_lowering=False, debug=False, num_devices=N_CORES)

    # x is host-duplicated: channels 0-63 = padded x, channels 64-127 = the
    # same image pre-shifted one column left (so kw=0 and kw=1 contract in a
    # single K=128 matmul; kw=2 uses the unshifted half at col offset 2).
    x_ap = nc.dram_tensor("x", [B_LOC, 2 * C_IN, 33, 2, WP], dt_mm, kind="ExternalInput").ap()
    h_ap = nc.dram_tensor("h", [B_LOC, C_HID, 33, 2, WP], dt_mm, kind="ExternalInput").ap()
    c_ap = nc.dram_tensor("c", [B_LOC, C_HID, NT, 2, W], dt_mm, kind="ExternalInput").ap()
    # Winograd-G-transformed weights: whw[u*3+kw] (K=128), wxw[u] (K=128,
    # stacked kw0|kw1), wx3[u] (K=64, kw2)
    whw_ap = nc.dram_tensor("whw", [12, C_HID, 4 * C_HID], dt_mm, kind="ExternalInput").ap()
    wxw_ap = nc.dram_tensor("wxw", [4, 2 * C_IN, 4 * C_HID], dt_mm, kind="ExternalInput").ap()
    wx3_ap = nc.dram_tensor("wx3", [4, C_IN, 4 * C_HID], dt_mm, kind="ExternalInput").ap()
    bias_ap = nc.dram_tensor("biasT", [C_HID, 4], f32, kind="ExternalInput").ap()
    # outputs laid out [tile, parity, col] so parity stores are strided APs
    hn_ap = nc.dram_tensor("hn", [B_LOC, C_HID, NT, 2, W], f32, kind="ExternalOutput").ap()
    cn_ap = nc.dram_tensor("cn", [B_LOC, C_HID, NT, 2, W], f32, kind="ExternalOutput").ap()

    SIG = mybir.ActivationFunctionType.Sigmoid
    TANH = mybir.ActivationFunctionType.Tanh
    COPY = mybir.ActivationFunctionType.Copy

    with tile.TileContext(nc) as tc:
        with (
            tc.tile_pool(name="weights", bufs=1) as wpool,
            tc.tile_pool(name="imgs", bufs=1) as ipool,
            tc.tile_pool(name="trans", bufs=1) as tpool,
            tc.tile_pool(name="cstate", bufs=3) as cpool,
            tc.tile_pool(name="psum", bufs=8, space="PSUM") as ppool,
            tc.tile_pool(name="acts", bufs=3) as apool,
            tc.tile_pool(name="gates", bufs=2) as gpool,
            tc.tile_pool(name="outs", bufs=3) as opool,
        ):
            whw_t = wpool.tile([C_HID, 12, 4 * C_HID], dt_mm, tag="whw")
            wxw_t = wpool.tile([2 * C_IN, 4, 4 * C_HID], dt_mm, tag="wxw")
            wx3_t = wpool.tile([C_IN, 4, 4 * C_HID], dt_mm, tag="wx3")
            bias_t = wpool.tile([C_HID, 4], f32, tag="bias")
            nc.sync.dma_start(whw_t[:], whw_ap.rearrange("t k m -> k t m"))
            nc.sync.dma_start(wxw_t[:], wxw_ap.rearrange("t k m -> k t m"))
            nc.sync.dma_start(wx3_t[:], wx3_ap.rearrange("t k m -> k t m"))
            nc.sync.dma_start(bias_t[:], bias_ap[:])

            # padded images, viewed as [33 row-pairs, 2, WP] for the strided
            # row access in the Winograd input transform
            hp = []
            xp = []
            for b in range(B_LOC):
                hp_b = ipool.tile([C_HID, 33, 2, WP], dt_mm, tag=f"hp{b}")
                xp_b = ipool.tile([2 * C_IN, 33, 2, WP], dt_mm, tag=f"xp{b}")
                nc.sync.dma_start(hp_b[:], h_ap[b])
                nc.sync.dma_start(xp_b[:], x_ap[b])
                hp.append(hp_b)
                xp.append(xp_b)

            # B^T row combos: t0=d0-d2, t1=d1+d2, t2=d2-d1, t3=d1-d3 where
            # d_u = padded row 2t+u of each row-tile t
            def transform(img_t, out_t, rep, bname):
                # img_t: [K, 33, 2, WP]; out_t: [K, 4, NT, WP]
                d0 = img_t[:, 0:NT, 0, :]
                d1 = img_t[:, 0:NT, 1, :]
                d2 = img_t[:, 1 : NT + 1, 0, :]
                d3 = img_t[:, 1 : NT + 1, 1, :]
                nc.vector.tensor_sub(out_t[:, 0], d0, d2)
                nc.vector.tensor_add(out_t[:, 1], d1, d2)
                nc.vector.tensor_sub(out_t[:, 2], d2, d1)
                nc.vector.tensor_sub(out_t[:, 3], d1, d3)

            for _rep in range(unroll):
                th = []
                tx = []
                for b in range(B_LOC):
                    th_b = tpool.tile([C_HID, 4, NT, WP], dt_mm, tag=f"th{b}", name=f"th{b}_{_rep}")
                    tx_b = tpool.tile([2 * C_IN, 4, NT, WP], dt_mm, tag=f"tx{b}", name=f"tx{b}_{_rep}")
                    transform(hp[b], th_b, _rep, b)
                    transform(xp[b], tx_b, _rep, b)
                    th.append(th_b)
                    tx.append(tx_b)

                for b in range(B_LOC):
                    for ch in range(NCHUNK):
                        t0 = ch * TPC
                        c_t = cpool.tile([C_HID, TPC, 2, W], dt_mm, tag="c", name=f"c_{_rep}_{b}_{ch}")
                        nc.sync.dma_start(c_t[:], c_ap[b][:, t0 : t0 + TPC])
                        ge = []
                        go = []
                        for g in range(4):
                            m = {}
                            # fill order m1, m2 first so their ACT copies
                            # overlap the m0/m3 matmuls
                            for u in (1, 2, 0, 3):
                                acc = ppool.tile([C_HID, TPC * W], mybir.dt.float32, tag="m")
                                for kw in range(3):
                                    nc.tensor.matmul(
                                        acc[:],
                                        whw_t[:, u * 3 + kw, g * C_HID : (g + 1) * C_HID],
                                        th[b][:, u, t0 : t0 + TPC, kw : kw + W],
                                        start=(kw == 0),
                                        stop=False,
                                    )
                                nc.tensor.matmul(
                                    acc[:],
                                    wxw_t[:, u, g * C_HID : (g + 1) * C_HID],
                                    tx[b][:, u, t0 : t0 + TPC, 0:W],
                                    start=False,
                                    stop=False,
                                )
                                nc.tensor.matmul(
                                    acc[:],
                                    wx3_t[:, u, g * C_HID : (g + 1) * C_HID],
                                    tx[b][0:C_IN, u, t0 : t0 + TPC, 2 : 2 + W],
                                    start=False,
                                    stop=True,
                                )
                                m[u] = acc
                            a_t = apool.tile([C_HID, TPC * W], dt_mm, tag="a")
                            b_t = apool.tile([C_HID, TPC * W], dt_mm, tag="b")
                            nc.scalar.activation(a_t[:], m[1][:], COPY)
                            nc.scalar.activation(b_t[:], m[2][:], COPY)
                            tp_t = apool.tile([C_HID, TPC * W], dt_mm, tag="tp")
                            tm_t = apool.tile([C_HID, TPC * W], dt_mm, tag="tm")
                            nc.vector.tensor_add(tp_t[:], a_t[:], b_t[:])
                            nc.vector.tensor_sub(tm_t[:], a_t[:], b_t[:])
                            ye_t = apool.tile([C_HID, TPC * W], mybir.dt.float32, tag="ye")
                            yo_t = apool.tile([C_HID, TPC * W], mybir.dt.float32, tag="yo")
                            nc.vector.tensor_add(ye_t[:], tp_t[:], m[0][:])
                            nc.vector.tensor_sub(yo_t[:], tm_t[:], m[3][:])
                            func = TANH if g == 3 else SIG
                            ge_t = gpool.tile([C_HID, TPC * W], dt_mm, tag=f"ge{g}")
                            go_t = gpool.tile([C_HID, TPC * W], dt_mm, tag=f"go{g}")
                            nc.scalar.activation(ge_t[:], ye_t[:], func, bias=bias_t[:, g : g + 1])
                            nc.scalar.activation(go_t[:], yo_t[:], func, bias=bias_t[:, g : g + 1])
                            ge.append(ge_t)
                            go.append(go_t)

                        for par, gates in ((0, ge), (1, go)):
                            i_t, f_t, o_t, g_t = gates
                            c_par = c_t[:, :, par, :]
                            ig = opool.tile([C_HID, TPC * W], dt_mm, tag="ig")
                            nc.vector.tensor_mul(ig[:], i_t[:], g_t[:])
                            fc = opool.tile([C_HID, TPC * W], dt_mm, tag="fc")
                            nc.vector.tensor_mul(fc[:], f_t[:], c_par)
                            cn_t = opool.tile([C_HID, TPC * W], mybir.dt.float32, tag="cn")
                            nc.vector.tensor_add(cn_t[:], fc[:], ig[:])
                            nc.sync.dma_start(cn_ap[b][:, t0 : t0 + TPC, par, :], cn_t[:])
                            th_t = opool.tile([C_HID, TPC * W], mybir.dt.float32, tag="th")
                            nc.scalar.activation(th_t[:], cn_t[:], TANH)
                            hn_t = opool.tile([C_HID, TPC * W], mybir.dt.float32, tag="hn")
                            nc.vector.tensor_mul(hn_t[:], o_t[:], th_t[:])
                            nc.sync.dma_start(hn_ap[b][:, t0 : t0 + TPC, par, :], hn_t[:])

    nc.compile()
    _cache[key] = nc
    return nc


def _prep_inputs(x, h_cur, c_cur, weight, bias, dt_mm):
    """Host-side transform/shard. Returns in_maps for the 8 cores."""
    if dt_mm == mybir.dt.bfloat16:
        import ml_dtypes

        npdt = ml_dtypes.bfloat16
    elif dt_mm == mybir.dt.float16:
        npdt = np.float16
    else:
        npdt = np.float32
    cast = lambda a: np.ascontiguousarray(a.astype(np.float32), dtype=None).astype(npdt)

    # weight: [4*C_HID, C_IN + C_HID, 3, 3] -> [kh, kw, ci, co]
    wt = np.ascontiguousarray(weight.transpose(2, 3, 1, 0)).astype(np.float32)
    wx = wt[:, :, :C_IN, :]  # [3,3,64,512]
    wh = wt[:, :, C_IN:, :]  # [3,3,128,512]
    # Winograd F(2,3) weight transform along kh: G rows
    Gc = np.array([[1, 0, 0], [0.5, 0.5, 0.5], [0.5, -0.5, 0.5], [0, 0, 1]], np.float32)
    whu = np.einsum("uk,kwco->uwco", Gc, wh)  # [4,3,128,512]
    wxu = np.einsum("uk,kwco->uwco", Gc, wx)  # [4,3,64,512]
    whw = cast(whu.reshape(12, C_HID, 4 * C_HID))
    wxw = cast(np.concatenate([wxu[:, 0], wxu[:, 1]], axis=1))  # [4,128,512]
    wx3 = cast(np.ascontiguousarray(wxu[:, 2]))  # [4,64,512]
    biasT = np.ascontiguousarray(bias.reshape(4, C_HID).T, dtype=np.float32)
    c3 = c_cur.reshape(B, C_HID, NT, 2, W).astype(npdt)

    # zero-pad x/h on host; x channels 64-127 hold the image shifted one
    # column left so taps (kh,0) and (kh,1) contract in one K=128 matmul.
    xpad = np.zeros((B, 2 * C_IN, HP, WP), dtype=np.float32)
    xpad[:, :C_IN, 1 : H + 1, 1 : W + 1] = x
    xpad[:, C_IN:, :, : WP - 1] = xpad[:, :C_IN, :, 1:]
    hpad = np.zeros((B, C_HID, HP, WP), dtype=np.float32)
    hpad[:, :, 1 : H + 1, 1 : W + 1] = h_cur
    xpad = xpad.astype(npdt).reshape(B, 2 * C_IN, 33, 2, WP)
    hpad = hpad.astype(npdt).reshape(B, C_HID, 33, 2, WP)

    in_maps = []
    for i in range(N_CORES):
        s = slice(i * B_LOC, (i + 1) * B_LOC)
        in_maps.append(
            {
                "x": xpad[s],
                "h": hpad[s],
                "c": c3[s],
                "whw": whw,
                "wxw": wxw,
                "wx3": wx3,
                "biasT": biasT,
            }
        )
    return in_maps


def run(x, h_cur, c_cur, weight, bias, dt_mm=mybir.dt.float16, trace=False):
    x = np.asarray(x)
    h_cur = np.asarray(h_cur)
    c_cur = np.asarray(c_cur)
    weight = np.asarray(weight)
    bias = np.asarray(bias)
    nc = _build(dt_mm, trace)
    in_maps = _prep_inputs(x, h_cur, c_cur, weight, bias, dt_mm)
    res = run_bass_kernel_spmd(nc, in_maps, list(range(N_CORES)), trace=trace)
    hn = np.concatenate([res.results[i]["hn"] for i in range(N_CORES)], axis=0)
    cn = np.concatenate([res.results[i]["cn"] for i in range(N_CORES)], axis=0)
    hn = hn.reshape(B, C_HID, H, W).astype(np.float32)
    cn = cn.reshape(B, C_HID, H, W).astype(np.float32)
    return (hn, cn), res


def kernel(x, h_cur, c_cur, weight, bias):
    (hn, cn), _ = run(x, h_cur, c_cur, weight, bias)
    return hn, cn


def _make_timing_fn(nc, in_maps):
    """Non-donating jitted runner with device-resident inputs, for
    throughput timing (slope of wall time vs iteration count)."""
    import jax
    from jax.sharding import NamedSharding

    from concourse import bass2jax, mybir as _mybir

    bass2jax.install_neuronx_cc_hook()
    n_cores = len(in_maps)
    partition_name = nc.partition_id_tensor.name if nc.partition_id_tensor else None
    in_names, out_names, out_avals, zero_outs = [], [], [], []
    for alloc in nc.m.functions[0].allocations:
        if not isinstance(alloc, _mybir.MemoryLocationSet):
            continue
        name = alloc.memorylocations[0].name
        if alloc.kind == "ExternalInput":
            if name != partition_name:
                in_names.append(name)
        elif alloc.kind == "ExternalOutput":
            out_names.append(name)
            shape = tuple(alloc.tensor_shape)
            dtype = _mybir.dt.np(alloc.dtype)
            out_avals.append(jax.core.ShapedArray(shape, dtype))
            zero_outs.append(np.zeros(shape, dtype))
    n_params = len(in_names)
    all_in_names = list(in_names) + list(out_names)
    if partition_name is not None:
        all_in_names.append(partition_name)

    def _body(*args):
        operands = list(args)
        if partition_name is not None:
            operands.append(bass2jax.partition_id_tensor())
        outs = bass2jax._bass_exec_p.bind(
            *operands,
            out_avals=tuple(out_avals),
            in_names=tuple(all_in_names),
            out_names=tuple(out_names),
            lowering_input_output_aliases=(),
            sim_require_finite=True,
            sim_require_nnan=True,
            nc=nc,
        )
        return tuple(outs)

    devices = jax.devices()[:n_cores]
    mesh = bass2jax.Mesh(np.asarray(devices), ("core",))
    in_specs = (bass2jax.PartitionSpec("core"),) * (n_params + len(out_names))
    out_specs = (bass2jax.PartitionSpec("core"),) * len(out_names)
    fn = jax.jit(
        bass2jax.shard_map(
            _body, mesh=mesh, in_specs=in_specs, out_specs=out_specs, check_rep=False
        ),
        keep_unused=True,
    )
    per_core = [[np.asarray(m[name]) for name in in_names] for m in in_maps]
    concat_in = [
        np.concatenate([per_core[c][i] for c in range(n_cores)], axis=0)
        for i in range(n_params)
    ]
    concat_zeros = [
        np.zeros((n_cores * z.shape[0], *z.shape[1:]), z.dtype) for z in zero_outs
    ]
    sh = NamedSharding(mesh, bass2jax.PartitionSpec("core"))
    dev_args = [jax.device_put(a, sh) for a in concat_in + concat_zeros]
    return fn, dev_args


def bench(x, h_cur, c_cur, weight, bias, dt_mm=None, ks=(4, 16)):
    """Returns estimated per-call device exec time in ns (pipelined slope)."""
    import time as _time

    import jax

    if dt_mm is None:
        dt_mm = mybir.dt.float16
    nc = _build(dt_mm)
    in_maps = _prep_inputs(
        np.asarray(x), np.asarray(h_cur), np.asarray(c_cur), np.asarray(weight), np.asarray(bias), dt_mm
    )
    fn, dev_args = _make_timing_fn(nc, in_maps)
    # warmup (compile + first exec)
    for _ in range(2):
        outs = fn(*dev_args)
        jax.block_until_ready(outs)

    def timed(k):
        t0 = _time.perf_counter()
        outs = None
        for _ in range(k):
            outs = fn(*dev_args)
        jax.block_until_ready(outs)
        return _time.perf_counter() - t0

    times = {}
    for k in ks:
        times[k] = min(timed(k) for _ in range(3))
    k_lo, k_hi = min(ks), max(ks)
    slope = (times[k_hi] - times[k_lo]) / (k_hi - k_lo)
    return slope * 1e9, times

